# revision 1
# baseline (speedup 1.0000x reference)
"""Trainium2 Bass kernel for nn_LSTMActor: trunk GEMM -> LayerNorm -> Tanh ->
LSTM (16 steps, constant input) -> MLP head -> tanh.

Sharding: data-parallel over batch B=2048 across 8 cores (256 rows each);
all weights replicated. Everything after the trunk runs in a transposed
layout (feature dim on partitions) so no per-step transposes are needed.
"""

import numpy as np
import ml_dtypes

import concourse.bass as bass
import concourse.tile as tile
from concourse import mybir, bacc
from concourse import bass_utils
from concourse.masks import make_identity

BF = ml_dtypes.bfloat16
F32 = mybir.dt.float32
BF16 = mybir.dt.bfloat16

B, R, Fd, H, A, T = 2048, 39200, 1024, 1024, 6, 16
NC_ = 8
BS = B // NC_          # 256 rows per core
NB = BS // 128         # 2 b-tiles per core
KT = 128               # contraction tile
RP = ((R + KT - 1) // KT) * KT   # 39296, padded R
NK = RP // KT          # 307 K-tiles for trunk
KH = H // 128          # 8 K-tiles for H-dim GEMMs
M4 = 4 * H // 128      # 32 M-tiles of gates
H2 = H // 2            # 512
KG = 2                 # trunk K-tiles per DMA batch (256KB wtr + 64KB obsT)

_CACHE = {}


def _build():
    nc = bacc.Bacc("TRN2", target_bir_lowering=False, debug=False)

    obsT_d = nc.dram_tensor("obsT", [RP, BS], BF16, kind="ExternalInput")
    wtr_d = nc.dram_tensor("wtr", [RP, Fd], BF16, kind="ExternalInput")
    wih_d = nc.dram_tensor("wih", [M4, 128, KH * 128], BF16, kind="ExternalInput")
    whh_d = nc.dram_tensor("whh", [H, 4 * H], BF16, kind="ExternalInput")
    w1_d = nc.dram_tensor("w1", [H, H2], BF16, kind="ExternalInput")
    w2_d = nc.dram_tensor("w2", [H2, A], BF16, kind="ExternalInput")
    btr_d = nc.dram_tensor("btr", [Fd], F32, kind="ExternalInput")
    gam_d = nc.dram_tensor("gam", [Fd], F32, kind="ExternalInput")
    bet_d = nc.dram_tensor("bet", [Fd], F32, kind="ExternalInput")
    bsum_d = nc.dram_tensor("bsum", [4 * H], F32, kind="ExternalInput")
    b1_d = nc.dram_tensor("b1", [H2], F32, kind="ExternalInput")
    b2_d = nc.dram_tensor("b2", [A], F32, kind="ExternalInput")
    mu_d = nc.dram_tensor("mu", [BS, T * A], F32, kind="ExternalOutput")

    AF = mybir.ActivationFunctionType

    def bc(ap1d, p=128):
        return bass.AP(tensor=ap1d.tensor, offset=ap1d.offset,
                       ap=[[0, p]] + [list(x) for x in ap1d.ap])

    with tile.TileContext(nc) as tc:
        with (
            tc.tile_pool(name="const", bufs=1) as cst,
            tc.tile_pool(name="state", bufs=1) as st,
            tc.tile_pool(name="wstream", bufs=2) as ws,
            tc.tile_pool(name="work", bufs=1) as wk,
            tc.tile_pool(name="acts", bufs=2) as ac,
        ):
            # ---- resident constants ----
            ident = cst.tile([128, 128], BF16)
            make_identity(nc, ident)
            whh_sb = cst.tile([128, KH, 4 * H], BF16)     # 64KB/part
            w1_sb = cst.tile([128, KH, H2], BF16)         # 8KB/part
            w2_sb = cst.tile([128, H2 // 128, A], BF16)   # [128,4,6]
            btr_b = cst.tile([128, Fd], F32)
            gam_b = cst.tile([128, Fd], F32)
            bet_b = cst.tile([128, Fd], F32)
            bsum_sb = cst.tile([128, M4], F32)            # [128,32] col m = bsum[m*128+p]
            nc.sync.dma_start(bsum_sb, bsum_d.ap().rearrange("(m p) -> p m", p=128))
            b1_sb = cst.tile([128, H2 // 128], F32)       # [128,4]
            nc.sync.dma_start(b1_sb, b1_d.ap().rearrange("(m p) -> p m", p=128))
            b2_b = cst.tile([128, A], F32)
            nc.sync.dma_start(b2_b, bc(b2_d.ap()))
            eps_t = cst.tile([128, 1], F32)
            nc.vector.memset(eps_t, 1e-5)

            # ---- persistent state ----
            xT = st.tile([128, KH, BS], BF16)             # x^T  [Fd, BS]
            preT = st.tile([128, M4, BS], BF16)            # pre^T [4H, BS] 32KB/part
            c_st = st.tile([128, KH, BS], F32)            # c^T  [H, BS]
            hT = [st.tile([128, KH, BS], BF16, name=f"hT{i}", tag=f"h{i}") for i in range(2)]  # ping-pong
            mu_sb = st.tile([128, NB, T * A], F32)        # [128,2,96]

            wtr_r = wtr_d.ap().rearrange("(ko p) n -> p ko n", p=128)
            obsT_r = obsT_d.ap().rearrange("(ko p) b -> p ko b", p=128)

            # ================= Phase 1: trunk GEMM =================
            with tc.tile_pool(name="ps_trunk", bufs=1, space="PSUM") as pst:
                psx = pst.tile([128, NB, Fd], F32)        # 8KB/part = 4 banks
                for kg in range(0, NK, KG):
                    kn = min(KG, NK - kg)
                    wt = ws.tile([128, KG, Fd], BF16, tag="wtr", bufs=6)
                    ot = ws.tile([128, KG, BS], BF16, tag="obsT", bufs=4)
                    nc.sync.dma_start(wt[:, :kn, :], wtr_r[:, kg : kg + kn, :])
                    nc.sync.dma_start(ot[:, :kn, :], obsT_r[:, kg : kg + kn, :])
                    for kk in range(kn):
                        k = kg + kk
                        for b in range(NB):
                            lhsT = ot[:, kk, b * 128 : (b + 1) * 128]
                            for n in range(2):
                                nc.tensor.matmul(
                                    psx[:, b, n * 512 : (n + 1) * 512],
                                    lhsT,
                                    wt[:, kk, n * 512 : (n + 1) * 512],
                                    start=(k == 0),
                                    stop=(k == NK - 1),
                                )

                # LN constants: needed right after trunk; emitted here so the
                # trunk's first chunks aren't queued behind them
                nc.sync.dma_start(btr_b, bc(btr_d.ap()))
                nc.sync.dma_start(gam_b, bc(gam_d.ap()))
                nc.sync.dma_start(bet_b, bc(bet_d.ap()))

                # ============ Phase 2: LayerNorm + tanh ============
                xa = wk.tile([128, NB, Fd], BF16, tag="xa")
                for b in range(NB):
                    xs = wk.tile([128, Fd], F32, tag="xs", bufs=2)
                    nc.vector.tensor_add(xs, psx[:, b, :], btr_b)
                    stats = wk.tile([128, 2, 6], F32, tag="stats")
                    for s in range(2):
                        nc.vector.bn_stats(
                            out=stats[:, s, :], in_=xs[:, s * 512 : (s + 1) * 512]
                        )
                    mv = wk.tile([128, 2], F32, tag="mv")
                    nc.vector.bn_aggr(out=mv, in_=stats)
                    rstd = wk.tile([128, 1], F32, tag="rstd")
                    nc.scalar.activation(
                        out=rstd, in_=mv[:, 1:2], func=AF.Sqrt, bias=eps_t, scale=1.0
                    )
                    nc.vector.reciprocal(out=rstd, in_=rstd)
                    nc.vector.scalar_tensor_tensor(
                        out=xs, in0=xs, scalar=mv[:, 0:1], in1=gam_b,
                        op0=mybir.AluOpType.subtract, op1=mybir.AluOpType.mult,
                    )
                    nc.vector.scalar_tensor_tensor(
                        out=xs, in0=xs, scalar=rstd, in1=bet_b,
                        op0=mybir.AluOpType.mult, op1=mybir.AluOpType.add,
                    )
                    nc.scalar.activation(out=xa[:, b, :], in_=xs, func=AF.Tanh)

            # ============ Phase 3: transpose x -> xT (bf16) ============
            with tc.tile_pool(name="ps_tr", bufs=4, space="PSUM") as ptr:
                for b in range(NB):
                    for f in range(KH):
                        pt = ptr.tile([128, 128], BF16, tag="tr")
                        nc.tensor.transpose(
                            pt, xa[:, b, f * 128 : (f + 1) * 128], ident
                        )
                        nc.scalar.activation(
                            out=xT[:, f, b * 128 : (b + 1) * 128], in_=pt, func=AF.Copy
                        )

            # ============ Phase 4: pre^T = W_ih^T x^T + bsum ============
            with tc.tile_pool(name="ps_pre", bufs=2, space="PSUM") as ppr:
                whh_r = whh_d.ap().rearrange("(ko p) n -> p ko n", p=128)
                for m in range(M4):
                    wm = ws.tile([128, KH, 128], BF16, tag="wih", bufs=4)
                    nc.sync.dma_start(
                        wm, wih_d.ap()[m].rearrange("p (k j) -> p k j", j=128))
                    if m % 4 == 0:
                        k8 = m // 4
                        nc.sync.dma_start(whh_sb[:, k8, :], whh_r[:, k8, :])
                        if k8 == 0:
                            nc.sync.dma_start(
                                w1_sb, w1_d.ap().rearrange("(ko p) n -> p ko n", p=128))
                            nc.sync.dma_start(
                                w2_sb, w2_d.ap().rearrange("(ko p) n -> p ko n", p=128))
                    ps = ppr.tile([128, BS], F32, tag="pre")
                    for k in range(KH):
                        nc.tensor.matmul(
                            ps, wm[:, k, :], xT[:, k, :],
                            start=(k == 0), stop=(k == KH - 1),
                        )
                    nc.vector.tensor_scalar_add(
                        preT[:, m, :], ps, bsum_sb[:, m : m + 1]
                    )

            # ============ Phase 5: LSTM steps ============
            with (
                tc.tile_pool(name="ps_g", bufs=5, space="PSUM") as psg,
                tc.tile_pool(name="ps_m", bufs=2, space="PSUM") as psm,
                tc.tile_pool(name="ps_w2", bufs=1, space="PSUM") as psw,
            ):
                relu1T = st.tile([128, H2 // 128, BS], BF16)

                def cell_update(j, si, sf, tg, so, first):
                    """c[j] = sf*c[j] + si*tg ; h[j] = so*tanh(c[j]) -> h_new."""
                    if first:
                        nc.vector.tensor_mul(c_st[:, j, :], si, tg)
                    else:
                        t1 = ac.tile([128, BS], F32, tag="t1")
                        nc.vector.tensor_mul(t1, si, tg)
                        nc.vector.tensor_mul(c_st[:, j, :], c_st[:, j, :], sf)
                        nc.vector.tensor_add(c_st[:, j, :], c_st[:, j, :], t1)
                    tcn = ac.tile([128, BS], F32, tag="tc")
                    nc.scalar.activation(out=tcn, in_=c_st[:, j, :], func=AF.Tanh)
                    nc.vector.tensor_mul(h_new[:, j, :], so, tcn)

                def mlp_head(t, h_cur):
                    for m in range(H2 // 128):
                        ps = psm.tile([128, BS], F32, tag="m1")
                        for k in range(KH):
                            nc.tensor.matmul(
                                ps, w1_sb[:, k, m * 128 : (m + 1) * 128],
                                h_cur[:, k, :],
                                start=(k == 0), stop=(k == KH - 1),
                            )
                        nc.scalar.activation(
                            out=relu1T[:, m, :], in_=ps, func=AF.Relu,
                            bias=b1_sb[:, m : m + 1], scale=1.0,
                        )
                    for b in range(NB):
                        ps2 = psw.tile([128, A], F32, tag="w2")
                        for k2 in range(H2 // 128):
                            nc.tensor.matmul(
                                ps2,
                                relu1T[:, k2, b * 128 : (b + 1) * 128],
                                w2_sb[:, k2, :],
                                start=(k2 == 0), stop=(k2 == H2 // 128 - 1),
                            )
                        t6 = ac.tile([128, A], F32, tag="t6")
                        nc.vector.tensor_add(t6, ps2, b2_b)
                        nc.scalar.activation(
                            out=mu_sb[:, b, t * A : (t + 1) * A], in_=t6, func=AF.Tanh
                        )

                # ---- step 0: h0 = c0 = 0 -> gates = pre ----
                h_new = hT[0]
                for j in range(KH):
                    si = ac.tile([128, BS], F32, tag="a0")
                    tg = ac.tile([128, BS], F32, tag="a2")
                    so = ac.tile([128, BS], F32, tag="a3")
                    nc.scalar.activation(out=si, in_=preT[:, j, :], func=AF.Sigmoid)
                    nc.scalar.activation(out=tg, in_=preT[:, 16 + j, :], func=AF.Tanh)
                    nc.scalar.activation(out=so, in_=preT[:, 24 + j, :], func=AF.Sigmoid)
                    cell_update(j, si, None, tg, so, first=True)
                mlp_head(0, hT[0])

                # ---- steps 1..15 ----
                for t in range(1, T):
                    h_cur = hT[(t + 1) % 2]
                    h_new = hT[t % 2]
                    for j in range(KH):
                        acts = {}
                        for q in range(4):
                            m = 8 * q + j
                            ps = psg.tile([128, BS], F32, tag="g")
                            for kk in range(KH):
                                k = (kk + j) % KH
                                nc.tensor.matmul(
                                    ps,
                                    whh_sb[:, k, m * 128 : (m + 1) * 128],
                                    h_cur[:, k, :],
                                    start=(kk == 0), stop=(kk == KH - 1),
                                )
                            tmp = ac.tile([128, BS], F32, tag=f"q{q}")
                            nc.vector.tensor_add(tmp, ps, preT[:, m, :])
                            out_a = ac.tile([128, BS], F32, tag=f"a{q}")
                            nc.scalar.activation(
                                out=out_a, in_=tmp,
                                func=AF.Tanh if q == 2 else AF.Sigmoid,
                            )
                            acts[q] = out_a
                        cell_update(j, acts[0], acts[1], acts[2], acts[3], first=False)
                    mlp_head(t, h_new)

            # ---- write out ----
            nc.sync.dma_start(
                mu_d.ap().rearrange("(bt p) f -> p bt f", p=128), mu_sb
            )

    nc.compile()
    return nc


def kernel(**inputs):
    obs = np.asarray(inputs["obs"], np.float32)
    W_trunk = np.asarray(inputs["W_trunk"], np.float32)
    b_trunk = np.asarray(inputs["b_trunk"], np.float32)
    gamma = np.asarray(inputs["gamma"], np.float32)
    beta = np.asarray(inputs["beta"], np.float32)
    W_ih = np.asarray(inputs["W_ih"], np.float32)
    b_ih = np.asarray(inputs["b_ih"], np.float32)
    W_hh = np.asarray(inputs["W_hh"], np.float32)
    b_hh = np.asarray(inputs["b_hh"], np.float32)
    W1 = np.asarray(inputs["W1"], np.float32)
    b1 = np.asarray(inputs["b1"], np.float32)
    W2 = np.asarray(inputs["W2"], np.float32)
    b2 = np.asarray(inputs["b2"], np.float32)
    num_actions = int(np.asarray(inputs["num_actions"]))
    assert num_actions == T, f"kernel hardcodes T={T}, got {num_actions}"
    assert obs.shape == (B, R)

    if "nc" not in _CACHE:
        _CACHE["nc"] = _build()
    nc = _CACHE["nc"]

    wtr = np.zeros((RP, Fd), BF)
    wtr[:R] = W_trunk.astype(BF)
    wih = np.ascontiguousarray(
        W_ih.astype(BF).reshape(KH, 128, M4, 128).transpose(2, 1, 0, 3)
    ).reshape(M4, 128, KH * 128)
    whh = W_hh.astype(BF)
    w1 = W1.astype(BF)
    w2 = W2.astype(BF)
    bsum = (b_ih + b_hh).astype(np.float32)

    in_maps = []
    for i in range(NC_):
        sh = obs[i * BS : (i + 1) * BS]           # [256, R]
        obsT = np.zeros((RP, BS), BF)
        obsT[:R] = np.ascontiguousarray(sh.T).astype(BF)
        in_maps.append({
            "obsT": obsT, "wtr": wtr, "wih": wih, "whh": whh,
            "w1": w1, "w2": w2, "btr": b_trunk, "gam": gamma,
            "bet": beta, "bsum": bsum, "b1": b1, "b2": b2,
        })

    res = bass_utils.run_bass_kernel_spmd(
        nc, in_maps, core_ids=list(range(NC_)),
        trace=bool(int(__import__("os").environ.get("KTRACE", "0"))),
    )
    _CACHE["last_result"] = res
    out = np.concatenate(
        [res.results[i]["mu"].reshape(BS, T, A) for i in range(NC_)], axis=0
    )
    return out



# revision 10
# speedup vs baseline: 1.0724x; 1.0724x over previous
"""Trainium2 Bass kernel for nn_LSTMActor: trunk GEMM -> LayerNorm -> Tanh ->
LSTM (16 steps, constant input) -> MLP head -> tanh.

Sharding: data-parallel over batch B=2048 across 8 cores (256 rows each);
weights replicated. Everything runs in a transposed layout (feature dim on
partitions, batch on the free axis):

  - trunk computed directly as x^T = W_trunk^T @ obs^T in fp16
  - LayerNorm in transposed layout (partition reductions via ones-matmuls)
  - LSTM recurrence: i/f/o gate matmuls in fp8 e4m3 with DoubleRow perf mode
    (K=256 per instruction, ~1.7x bf16 rate); the g gate stays fp16 since its
    error feeds c undamped. h kept in fp16 (for g/W1) and scaled fp8 (for ifo).
  - gates evacuated per 4-m-tile groups so DVE/ACT ops are 1024-col wide
  - MLP head for step t runs pipelined inside step t+1's gate matmuls
"""

import numpy as np
import ml_dtypes

import concourse.bass as bass
import concourse.tile as tile
from concourse import mybir, bacc
from concourse import bass_utils

F8 = ml_dtypes.float8_e4m3fn
F32 = mybir.dt.float32
FP16 = mybir.dt.float16
BF16 = mybir.dt.bfloat16
FP8 = mybir.dt.float8e4

B, R, Fd, H, A, T = 2048, 39200, 1024, 1024, 6, 16
NC_ = 8
BS = B // NC_          # 256 rows per core
KT = 128
RP = ((R + KT - 1) // KT) * KT   # 39296
NK = RP // KT          # 307 K-tiles for trunk
KH = H // 128          # 8 k-tiles over H
H2 = H // 2            # 512
KG = 2                 # trunk K-tiles per DMA batch

S_W = 256.0            # fp8 scale for W_hh (ifo cols)
S_H = 32.0             # fp8 scale for h
DQ = 1.0 / (S_W * S_H)

DR = mybir.MatmulPerfMode.DoubleRow

_CACHE = {}


def _build():
    nc = bacc.Bacc("TRN2", target_bir_lowering=False, debug=False)

    obsT_d = nc.dram_tensor("obsT", [RP, BS], FP16, kind="ExternalInput")
    wtr_d = nc.dram_tensor("wtr", [RP, Fd], FP16, kind="ExternalInput")
    wih_d = nc.dram_tensor("wih", [32, 128, KH * 128], FP16, kind="ExternalInput")
    whh8_d = nc.dram_tensor("whh8", [128, 4, 2, 3 * H], FP8, kind="ExternalInput")
    whhg_d = nc.dram_tensor("whhg", [128, KH, H], FP16, kind="ExternalInput")
    w1_d = nc.dram_tensor("w1", [128, KH, H2], FP16, kind="ExternalInput")
    w2_d = nc.dram_tensor("w2", [128, H2 // 128, A], FP16, kind="ExternalInput")
    btr_d = nc.dram_tensor("btr", [Fd], F32, kind="ExternalInput")
    gam_d = nc.dram_tensor("gam", [Fd], F32, kind="ExternalInput")
    bet_d = nc.dram_tensor("bet", [Fd], F32, kind="ExternalInput")
    bsum_d = nc.dram_tensor("bsum", [4 * H], FP16, kind="ExternalInput")
    b1_d = nc.dram_tensor("b1", [H2], F32, kind="ExternalInput")
    b2_d = nc.dram_tensor("b2", [A], F32, kind="ExternalInput")
    mu_d = nc.dram_tensor("mu", [A, T * BS], F32, kind="ExternalOutput")

    AF = mybir.ActivationFunctionType
    OP = mybir.AluOpType

    with tile.TileContext(nc) as tc:
        with (
            tc.tile_pool(name="const", bufs=1) as cst,
            tc.tile_pool(name="state", bufs=1) as st,
            tc.tile_pool(name="wstream", bufs=2) as ws,
            tc.tile_pool(name="acts", bufs=2) as ac,
        ):
            # ---- small resident constants ----
            ones_col = cst.tile([128, 1], BF16)          # lhsT for feature sums
            nc.vector.memset(ones_col, 1.0)
            ones_f32 = cst.tile([128, 128], F32)         # [0:1,:] lhsT for bcast
            nc.vector.memset(ones_f32[0:1, :], 1.0)
            ones_row = cst.tile([128, BS], FP16)         # [0:1,:] rhs for bsum init
            nc.vector.memset(ones_row[0:1, :], 1.0)
            eps_t = cst.tile([128, 1], F32)
            nc.vector.memset(eps_t, 1e-5)
            btr_t = cst.tile([128, KH], F32)
            nc.sync.dma_start(btr_t, btr_d.ap().rearrange("(m p) -> p m", p=128))
            gam_t = cst.tile([128, KH], F32)
            nc.sync.dma_start(gam_t, gam_d.ap().rearrange("(m p) -> p m", p=128))
            bet_t = cst.tile([128, KH], F32)
            nc.sync.dma_start(bet_t, bet_d.ap().rearrange("(m p) -> p m", p=128))
            bsum_row = cst.tile([128, 4 * H], FP16)      # [0:1,:]
            nc.sync.dma_start(
                bsum_row[0:1, :], bsum_d.ap().rearrange("(p x) -> p x", p=1))
            b1_t = cst.tile([128, H2 // 128], F32)
            nc.sync.dma_start(b1_t, b1_d.ap().rearrange("(m p) -> p m", p=128))
            b2_t = cst.tile([128, 1], F32)
            nc.sync.dma_start(b2_t[0:A, :], b2_d.ap().rearrange("(p x) -> p x", p=A))

            # ---- LSTM-phase resident weights (DMA'd near end of trunk) ----
            whh8 = cst.tile([128, 4, 2, 3 * H], FP8)     # 24KB/part
            whhg = cst.tile([128, KH, H], FP16)          # 16KB/part
            w1_sb = cst.tile([128, KH, H2], FP16)        # 8KB/part
            w2_sb = cst.tile([128, H2 // 128, A], FP16)

            # ---- persistent state ----
            preT = st.tile([128, 32, BS], BF16)          # pre^T [4H, BS] 16KB
            xa = st.tile([128, KH, BS], FP16)            # tanh(LN(x))^T 4KB
            c_st = st.tile([128, KH, BS], FP16)          # c^T
            h16 = [st.tile([128, KH, BS], FP16, name=f"h16_{i}") for i in range(2)]
            h8 = [st.tile([128, 4, 2, BS], FP8, name=f"h8_{i}") for i in range(2)]
            sig_q = {q: st.tile([128, KH, BS], BF16, name=f"sig{q}")
                     for q in range(4)}                  # sigma(i),sigma(f),tanh(g),sigma(o)
            t1 = st.tile([128, KH, BS], FP16)
            tcn = st.tile([128, KH, BS], FP16)
            muT = st.tile([128, T, BS], F32)             # [0:A] used

            wtr_r = wtr_d.ap().rearrange("(ko p) n -> p ko n", p=128)
            obsT_r = obsT_d.ap().rearrange("(ko p) b -> p ko b", p=128)

            # ================= Phase 1: trunk x^T = W^T obs^T =================
            # each m accumulation group owns a full 2KB PSUM bank: interleaved
            # start=True in a shared bank zeroes the bank-mate's partial sums
            with tc.tile_pool(name="ln", bufs=1) as ln:
                xs = ln.tile([128, KH, BS], F32)
                xsb = ln.tile([128, KH, BS], BF16)
                sq = ln.tile([128, KH, BS], BF16)
                with tc.tile_pool(name="ps_trunk", bufs=1, space="PSUM") as pst:
                    psx = pst.tile([128, KH, 512], F32)  # 16KB: bank per m
                    for kg in range(0, NK, KG):
                        kn = min(KG, NK - kg)
                        wt = ws.tile([128, KG, Fd], FP16, tag="wtr", bufs=6)
                        ot = ws.tile([128, KG, BS], FP16, tag="obsT", bufs=6)
                        nc.sync.dma_start(wt[:, :kn, :], wtr_r[:, kg:kg + kn, :])
                        nc.sync.dma_start(ot[:, :kn, :], obsT_r[:, kg:kg + kn, :])
                        for kk in range(kn):
                            k = kg + kk
                            for m in range(KH):
                                nc.tensor.matmul(
                                    psx[:, m, 0:BS],
                                    wt[:, kk, m * 128:(m + 1) * 128],
                                    ot[:, kk, :],
                                    start=(k == 0), stop=(k == NK - 1),
                                )
                    # queue LSTM weights behind the trunk stream; they land
                    # during LN/pre
                    nc.sync.dma_start(whh8, whh8_d.ap())
                    nc.sync.dma_start(whhg, whhg_d.ap())
                    nc.sync.dma_start(w1_sb, w1_d.ap())
                    nc.sync.dma_start(w2_sb, w2_d.ap())

                    for m in range(KH):
                        nc.scalar.activation(
                            out=xs[:, m, :], in_=psx[:, m, 0:BS], func=AF.Identity,
                            bias=btr_t[:, m:m + 1], scale=1.0)

                # ============ Phase 2: LayerNorm + tanh (transposed) ============
                with (
                    tc.tile_pool(name="ps_ln", bufs=1, space="PSUM") as pln,
                ):
                    nc.vector.tensor_copy(xsb, xs)
                    nc.vector.tensor_mul(sq, xs, xs)
                    ps_s = pln.tile([128, 2 * BS], F32)  # [0:1]: sum x | sum x^2
                    for m in range(KH):
                        nc.tensor.matmul(
                            ps_s[0:1, 0:BS], ones_col, xsb[:, m, :],
                            start=(m == 0), stop=(m == KH - 1))
                    for m in range(KH):
                        nc.tensor.matmul(
                            ps_s[0:1, BS:2 * BS], ones_col, sq[:, m, :],
                            start=(m == 0), stop=(m == KH - 1))
                    srow = ln.tile([128, 2 * BS], F32)   # [0:1]: mean | E[x^2]
                    nc.scalar.activation(
                        out=srow[0:1, :], in_=ps_s[0:1, :], func=AF.Copy,
                        scale=1.0 / Fd)
                    var = ln.tile([128, BS], F32)        # [0:1]
                    nc.vector.scalar_tensor_tensor(
                        out=var[0:1, :], in0=srow[0:1, 0:BS], scalar=-1.0,
                        in1=srow[0:1, 0:BS], op0=OP.mult, op1=OP.mult)
                    nc.vector.tensor_add(
                        var[0:1, :], srow[0:1, BS:2 * BS], var[0:1, :])
                    sd = ln.tile([128, BS], F32)
                    nc.scalar.activation(
                        out=sd[0:1, :], in_=var[0:1, :], func=AF.Sqrt,
                        bias=eps_t[0:1, :], scale=1.0)
                    srow2 = ln.tile([128, 2 * BS], F32)  # [0:1]: mean | rstd
                    nc.vector.reciprocal(out=srow2[0:1, BS:2 * BS], in_=sd[0:1, :])
                    nc.vector.tensor_copy(srow2[0:1, 0:BS], srow[0:1, 0:BS])
                    ps_b = pln.tile([128, 2 * BS], F32)  # bcast mean | rstd
                    nc.tensor.matmul(
                        ps_b, ones_f32[0:1, :], srow2[0:1, :], start=True, stop=True)
                    mb = ln.tile([128, 2 * BS], F32)
                    nc.scalar.activation(out=mb, in_=ps_b, func=AF.Copy, scale=1.0)

                    # x_norm = (xs - mean)*rstd*gamma + beta ; xa = tanh fp16
                    for m in range(KH):
                        nc.vector.tensor_sub(xs[:, m, :], xs[:, m, :], mb[:, 0:BS])
                        nc.vector.tensor_mul(xs[:, m, :], xs[:, m, :], mb[:, BS:2 * BS])
                        nc.vector.tensor_scalar(
                            out=xs[:, m, :], in0=xs[:, m, :],
                            scalar1=gam_t[:, m:m + 1], scalar2=bet_t[:, m:m + 1],
                            op0=OP.mult, op1=OP.add)
                    nc.scalar.activation(out=xa, in_=xs, func=AF.Tanh)

            # ============ Phase 3: pre^T = W_ih^T xa^T + bsum ============
            with tc.tile_pool(name="ps_pre", bufs=2, space="PSUM") as ppr:
                for q in range(4):
                    psq = ppr.tile([128, KH, BS], F32, tag="pre")
                    for mm in range(8):
                        m = q * 8 + mm
                        wm = ws.tile([128, KH, 128], FP16, tag="wih", bufs=4)
                        nc.sync.dma_start(
                            wm, wih_d.ap()[m].rearrange("p (k j) -> p k j", j=128))
                        nc.tensor.matmul(
                            psq[:, mm, :],
                            bsum_row[0:1, m * 128:(m + 1) * 128],
                            ones_row[0:1, :],
                            start=True, stop=False)
                        for k in range(KH):
                            nc.tensor.matmul(
                                psq[:, mm, :], wm[:, k, :], xa[:, k, :],
                                start=False, stop=(k == KH - 1))
                    nc.scalar.activation(
                        out=preT[:, q * 8:(q + 1) * 8, :], in_=psq, func=AF.Copy,
                        scale=1.0)

                    # ---- step 0 activations for this quarter (h0 = c0 = 0) ----
                    nc.scalar.activation(
                        out=sig_q[q], in_=preT[:, q * 8:(q + 1) * 8, :],
                        func=AF.Tanh if q == 2 else AF.Sigmoid)

                # step 0 cell: c = sig(i)*tanh(g); h = sig(o)*tanh(c)
                for hf in range(2):
                    sl = slice(hf * 4, hf * 4 + 4)
                    nc.vector.tensor_mul(c_st[:, sl, :], sig_q[0][:, sl, :],
                                         sig_q[2][:, sl, :])
                    nc.scalar.activation(out=tcn[:, sl, :], in_=c_st[:, sl, :],
                                         func=AF.Tanh)
                    nc.vector.tensor_mul(h16[0][:, sl, :], sig_q[3][:, sl, :],
                                         tcn[:, sl, :])
                    for u in range(2):
                        nc.scalar.activation(
                            out=h8[0][:, 2 * hf + u, :, :],
                            in_=h16[0][:, 4 * hf + 2 * u:4 * hf + 2 * u + 2, :],
                            func=AF.Copy, scale=S_H)

            # ============ Phase 4: LSTM steps 1..15 + pipelined MLP head ======
            with (
                tc.tile_pool(name="ps_g", bufs=2, space="PSUM") as psg,
                tc.tile_pool(name="ps_h", bufs=2, space="PSUM") as psh,
            ):
                relu1 = st.tile([128, H2 // 128, BS], FP16)

                def emit_head(t):
                    """MLP head on h16[t%2] -> muT[:, t, :]."""
                    hcur = h16[t % 2]
                    psw1 = psh.tile([128, 4, BS], F32, tag="w1", bufs=1)
                    for mm in range(4):
                        for k in range(KH):
                            nc.tensor.matmul(
                                psw1[:, mm, :],
                                w1_sb[:, k, mm * 128:(mm + 1) * 128],
                                hcur[:, k, :],
                                start=(k == 0), stop=(k == KH - 1))
                    for mm in range(4):
                        nc.scalar.activation(
                            out=relu1[:, mm, :], in_=psw1[:, mm, :], func=AF.Relu,
                            bias=b1_t[:, mm:mm + 1], scale=1.0)
                    psw2 = psh.tile([128, BS], F32, tag="w2", bufs=1)
                    for k2 in range(H2 // 128):
                        nc.tensor.matmul(
                            psw2[0:A, :], w2_sb[:, k2, :], relu1[:, k2, :],
                            start=(k2 == 0), stop=(k2 == H2 // 128 - 1))
                    nc.scalar.activation(
                        out=muT[0:A, t, :], in_=psw2[0:A, :], func=AF.Tanh,
                        bias=b2_t[0:A, :], scale=1.0)

                # quarter order: i(0), g(2), f(1), o(3)
                for t in range(1, T):
                    hp = h16[(t + 1) % 2]
                    h8p = h8[(t + 1) % 2]
                    hn = h16[t % 2]
                    h8n = h8[t % 2]
                    for qi, q in enumerate((0, 2, 1, 3)):
                        for hf in range(2):
                            sl = slice(hf * 4, hf * 4 + 4)
                            ps = psg.tile([128, 4, BS], F32, tag="gate")
                            if q != 2:
                                goff = {0: 0, 1: H, 3: 2 * H}[q]
                                for mm in range(4):
                                    col = goff + (hf * 4 + mm) * 128
                                    for kp in range(4):
                                        nc.tensor.matmul(
                                            ps[:, mm, :],
                                            whh8[:, kp, :, col:col + 128],
                                            h8p[:, kp, :, :],
                                            start=(kp == 0), stop=(kp == 3),
                                            perf_mode=DR)
                            else:
                                for mm in range(4):
                                    col = (hf * 4 + mm) * 128
                                    for k in range(KH):
                                        nc.tensor.matmul(
                                            ps[:, mm, :],
                                            whhg[:, k, col:col + 128],
                                            hp[:, k, :],
                                            start=(k == 0), stop=(k == KH - 1))
                            gq = ac.tile([128, 4, BS], BF16, tag="gq", bufs=3)
                            nc.vector.scalar_tensor_tensor(
                                out=gq, in0=ps, scalar=(DQ if q != 2 else 1.0),
                                in1=preT[:, q * 8 + hf * 4:q * 8 + hf * 4 + 4, :],
                                op0=OP.mult, op1=OP.add)
                            nc.scalar.activation(
                                out=sig_q[q][:, sl, :], in_=gq,
                                func=AF.Tanh if q == 2 else AF.Sigmoid)
                            # cell chain pieces as operands become ready
                            if q == 2:  # after tanh(g): t1 = sig(i)*tg
                                nc.vector.tensor_mul(
                                    t1[:, sl, :], sig_q[0][:, sl, :],
                                    sig_q[2][:, sl, :])
                            elif q == 1:  # after sig(f): c = c*sf + t1; tanh
                                nc.vector.tensor_mul(
                                    c_st[:, sl, :], c_st[:, sl, :],
                                    sig_q[1][:, sl, :])
                                nc.vector.tensor_add(
                                    c_st[:, sl, :], c_st[:, sl, :], t1[:, sl, :])
                                nc.scalar.activation(
                                    out=tcn[:, sl, :], in_=c_st[:, sl, :],
                                    func=AF.Tanh)
                            elif q == 3:  # after sig(o): h = so*tcn; h8
                                nc.vector.tensor_mul(
                                    hn[:, sl, :], sig_q[3][:, sl, :],
                                    tcn[:, sl, :])
                                for u in range(2):
                                    nc.scalar.activation(
                                        out=h8n[:, 2 * hf + u, :, :],
                                        in_=hn[:, 4 * hf + 2 * u:4 * hf + 2 * u + 2, :],
                                        func=AF.Copy, scale=S_H)
                    # pipelined MLP head of step t-1: its matmuls fill the PE
                    # bubble while this step's h propagates through DVE/ACT
                    emit_head(t - 1)
                emit_head(T - 1)

            # ---- write out ----
            nc.sync.dma_start(
                mu_d.ap().rearrange("a (t b) -> a t b", b=BS), muT[0:A, :, :])

    nc.compile()
    return nc


def kernel(**inputs):
    obs = np.asarray(inputs["obs"], np.float32)
    W_trunk = np.asarray(inputs["W_trunk"], np.float32)
    b_trunk = np.asarray(inputs["b_trunk"], np.float32)
    gamma = np.asarray(inputs["gamma"], np.float32)
    beta = np.asarray(inputs["beta"], np.float32)
    W_ih = np.asarray(inputs["W_ih"], np.float32)
    b_ih = np.asarray(inputs["b_ih"], np.float32)
    W_hh = np.asarray(inputs["W_hh"], np.float32)
    b_hh = np.asarray(inputs["b_hh"], np.float32)
    W1 = np.asarray(inputs["W1"], np.float32)
    b1 = np.asarray(inputs["b1"], np.float32)
    W2 = np.asarray(inputs["W2"], np.float32)
    b2 = np.asarray(inputs["b2"], np.float32)
    num_actions = int(np.asarray(inputs["num_actions"]))
    assert num_actions == T, f"kernel hardcodes T={T}, got {num_actions}"
    assert obs.shape == (B, R)

    if "nc" not in _CACHE:
        _CACHE["nc"] = _build()
    nc = _CACHE["nc"]

    wtr = np.zeros((RP, Fd), np.float16)
    wtr[:R] = W_trunk.astype(np.float16)
    wih = np.ascontiguousarray(
        W_ih.astype(np.float16).reshape(KH, 128, 32, 128).transpose(2, 1, 0, 3)
    ).reshape(32, 128, KH * 128)
    # whh8: [p, kp, half, 3H] fp8 for gates i,f,o ; whhg: [p, k, H] fp16 for g
    Wr = W_hh.reshape(4, 2, 128, 4 * H)     # [kp, half, p, 4H]
    ifo = np.concatenate([Wr[..., 0:H], Wr[..., H:2 * H], Wr[..., 3 * H:4 * H]],
                         axis=-1)           # [kp, half, p, 3H]
    whh8 = np.clip(ifo * S_W, -240, 240).astype(F8).transpose(2, 0, 1, 3)
    whh8 = np.ascontiguousarray(whh8)       # [128, 4, 2, 3H]
    whhg = np.ascontiguousarray(
        W_hh[:, 2 * H:3 * H].astype(np.float16).reshape(KH, 128, H).transpose(1, 0, 2))
    w1 = np.ascontiguousarray(
        W1.astype(np.float16).reshape(KH, 128, H2).transpose(1, 0, 2))
    w2 = np.ascontiguousarray(
        W2.astype(np.float16).reshape(H2 // 128, 128, A).transpose(1, 0, 2))
    bsum = (b_ih + b_hh).astype(np.float16)

    in_maps = []
    for i in range(NC_):
        sh = obs[i * BS:(i + 1) * BS]           # [256, R]
        obsT = np.zeros((RP, BS), np.float16)
        obsT[:R] = np.ascontiguousarray(sh.T).astype(np.float16)
        in_maps.append({
            "obsT": obsT, "wtr": wtr, "wih": wih, "whh8": whh8, "whhg": whhg,
            "w1": w1, "w2": w2, "btr": b_trunk, "gam": gamma,
            "bet": beta, "bsum": bsum, "b1": b1, "b2": b2,
        })

    res = bass_utils.run_bass_kernel_spmd(
        nc, in_maps, core_ids=list(range(NC_)),
        trace=bool(int(__import__("os").environ.get("KTRACE", "0"))),
    )
    _CACHE["last_result"] = res
    out = np.concatenate(
        [res.results[i]["mu"].reshape(A, T, BS).transpose(2, 1, 0)
         for i in range(NC_)], axis=0
    )
    return np.ascontiguousarray(out)
